# revision 8
# baseline (speedup 1.0000x reference)
"""GroupChat model (utterance/context/speaker/response GRUs) on 8 trn2 cores.

Data-parallel over batch (8 batches/core). All embedding gathers, GRU scans and
output transposes run on-device; host only shards inputs, packs weights
(transpose/reorder), builds gather indices, and unshards outputs.

Layout convention on device: all GRU states/gates are "feature-major":
a [128, 2*ST] SBUF tile where cols [0:N] hold feature rows 0:128 and cols
[ST:ST+N] hold feature rows 128:256 (ST = padded block stride).
Matmuls: out = lhsT.T @ rhs with lhsT = weight chunks [128,128] (host
pre-transposed), rhs = state/x column slices. Gate biases are folded in as
K=1 rank-1 matmuls against a constant ones row. The speaker GRU's masked
update (only the speaking agent's row changes per step) is implemented by
biasing the z-gate pre-activation with +30*(1-active): sigmoid saturates to
1 and the GRU step becomes the identity for inactive rows.
"""
import sys
sys.path.insert(0, '/opt/trn_rl_repo')
import numpy as np

B, S, T, R = 64, 50, 50, 2
V, H = 32000, 256
NCORES = 8
BL = B // NCORES          # 8 batches per core
NU = BL * S               # 400 utterance seqs / core
NRS = BL * R              # 16 response seqs / core
NRT = NRS * T             # 800 response tokens / core
NRTP = 896                # padded to 7*128
NSPK = BL * (S + 1)       # 408 speaker slots / core

F32 = None  # filled on first build
_cached = {}


def _agent_tables():
    np.random.seed(1)
    return np.stack([np.random.uniform(0, 1, (S + 1, H)) for _ in range(B)]).astype(np.float32)


def _pack_wT(W):
    """W [768, K] -> lhsT sbuf layout [128, (K/128)*768]: sb[p, kc*768+j] = W[j, kc*128+p]."""
    K = W.shape[1]
    kc = K // 128
    WT = np.ascontiguousarray(W.T)           # [K, 768]
    return WT.reshape(kc, 128, 768).transpose(1, 0, 2).reshape(128, kc * 768).copy()


def _pack_idx(idx_flat, npad):
    """idx [n] -> padded to npad -> [16, npad/16] with element i at [i%16, i//16]."""
    n = idx_flat.shape[0]
    out = np.zeros(npad, np.int16)
    out[:n] = idx_flat.astype(np.int16)
    return np.ascontiguousarray(np.tile(out.reshape(npad // 16, 16).T, (8, 1)))


def _build_nc():
    import concourse.bacc as bacc
    import concourse.mybir as mybir
    import concourse.tile as tile
    from concourse.masks import make_identity

    f32 = mybir.dt.float32
    f32r = mybir.dt.float32r
    mdt = f32r
    i16 = mybir.dt.int16
    SIG = mybir.ActivationFunctionType.Sigmoid
    TANH = mybir.ActivationFunctionType.Tanh
    CPY = mybir.ActivationFunctionType.Copy
    MUL = mybir.AluOpType.mult
    ADD = mybir.AluOpType.add
    SUB = mybir.AluOpType.subtract

    nc = bacc.Bacc("TRN2", target_bir_lowering=False, debug=False)

    # ---- DRAM tensors ----
    emb_u = nc.dram_tensor("emb_utter", [V, H], f32, kind="ExternalInput")
    emb_r = nc.dram_tensor("emb_resp", [V, H], f32, kind="ExternalInput")
    uidx_d = nc.dram_tensor("uidx", [128, T * 32], i16, kind="ExternalInput")
    ridx_d = nc.dram_tensor("ridx", [128, NRTP // 16], i16, kind="ExternalInput")
    aeT_d = nc.dram_tensor("aeT", [256, NU], f32, kind="ExternalInput")
    mask_d = nc.dram_tensor("mask", [T, NSPK], f32, kind="ExternalInput")
    wts_d = {}
    for g, kin in (("u", 256), ("c", 512), ("r", 256), ("s", 256)):
        wts_d[g + "_wihT"] = nc.dram_tensor(g + "_wihT", [128, (kin // 128) * 768], f32, kind="ExternalInput")
        wts_d[g + "_whhT"] = nc.dram_tensor(g + "_whhT", [128, 2 * 768], f32, kind="ExternalInput")
        wts_d[g + "_rzb"] = nc.dram_tensor(g + "_rzb", [1, 512], f32, kind="ExternalInput")
        wts_d[g + "_hnb"] = nc.dram_tensor(g + "_hnb", [1, 256], f32, kind="ExternalInput")
        wts_d[g + "_inb"] = nc.dram_tensor(g + "_inb", [1, 256], f32, kind="ExternalInput")
    ctxT_o = nc.dram_tensor("ctxT", [NU, 256], f32, kind="ExternalOutput")
    respT_o = nc.dram_tensor("respT", [NRT, 256], f32, kind="ExternalOutput")
    spkT_o = nc.dram_tensor("spkT", [NSPK, 256], f32, kind="ExternalOutput")

    r_ = lambda ap: ap.bitcast(f32r)

    with tile.TileContext(nc) as tc:
        with tc.tile_pool(name="const", bufs=1) as cp, \
             tc.tile_pool(name="work", bufs=2) as wp, \
             tc.tile_pool(name="psum", bufs=1, space="PSUM") as pp:

            # ---- persistent SBUF ----
            wt = {}
            for name, d in wts_d.items():
                shp = list(d.shape)
                t_ = cp.tile(shp, mdt, tag=name)
                nc.gpsimd.dma_start(t_[:], d[:])
                wt[name] = t_
            uidx = cp.tile([128, T * 32], i16, tag="uidx")
            nc.sync.dma_start(uidx[:], uidx_d[:])
            ridx = cp.tile([128, NRTP // 16], i16, tag="ridx")
            nc.sync.dma_start(ridx[:], ridx_d[:])
            aeT = cp.tile([128, 1024], mdt, tag="aeT")
            nc.gpsimd.dma_start(aeT[:, 0:NU], aeT_d[0:128, :])
            nc.gpsimd.dma_start(aeT[:, 512:512 + NU], aeT_d[128:256, :])

            ones = cp.tile([1, 512], mdt, tag="ones")
            nc.vector.memset(ones[:].bitcast(f32), 1.0)
            neg30 = cp.tile([1, 128], mdt, tag="neg30")
            nc.vector.memset(neg30[:].bitcast(f32), -30.0)
            ident = cp.tile([128, 128], f32, tag="ident")
            make_identity(nc, ident[:])
            ident_r = cp.tile([128, 128], mdt, tag="ident_r")
            nc.vector.tensor_copy(ident_r[:], ident[:])

            utterh = cp.tile([128, 1024], mdt, tag="utterh")     # final utter hidden
            ctx_out = cp.tile([128, 1024], mdt, tag="ctx_out")   # ctx outputs, halves @0/512
            resp_outs = cp.tile([128, 2 * NRTP], mdt, tag="resp_outs")  # halves @0/896
            resp_xT = cp.tile([128, 2 * NRTP], mdt, tag="resp_xT")
            zeros800 = cp.tile([128, 1024], mdt, tag="zeros800")
            nc.vector.memset(zeros800[:].bitcast(f32), 0.0)

            def gate_mm(dst, wihT, whhT, x_chunks, h_chunks, m, bias_sl, extra=None):
                """Emit all matmuls accumulating one gate m-chunk into psum dst."""
                first = True
                for kc, hc in enumerate(h_chunks):
                    nc.tensor.matmul(dst, whhT[:, kc * 768 + m * 128: kc * 768 + (m + 1) * 128],
                                     hc, start=first, stop=False)
                    first = False
                for kc, xc in enumerate(x_chunks):
                    nc.tensor.matmul(dst, wihT[:, kc * 768 + m * 128: kc * 768 + (m + 1) * 128],
                                     xc, start=first, stop=False)
                    first = False
                last = extra is None
                nc.tensor.matmul(dst, bias_sl, ones[0:1, 0:dst.shape[-1]],
                                 start=first, stop=last)
                if extra is not None:
                    lhs, rhs = extra
                    nc.tensor.matmul(dst, lhs, rhs, start=False, stop=True)

            def gru_step(cfg, x_chunks, h_merged, h_chunks, out_ap):
                """One GRU step. h_merged [128, 2N]; writes h' to out_ap [128, 2N]."""
                N, ST, tg = cfg["N"], cfg["st"], cfg["tag"]
                wihT, whhT = wt[cfg["g"] + "_wihT"], wt[cfg["g"] + "_whhT"]
                rzb, hnb, inb = wt[cfg["g"] + "_rzb"], wt[cfg["g"] + "_hnb"], wt[cfg["g"] + "_inb"]
                mrow = cfg.get("mask_row")
                rz = pp.tile([128, 4 * ST], f32, tag=tg + "rz")
                hn = pp.tile([128, 2 * ST], f32, tag=tg + "hn")
                inn = pp.tile([128, 2 * ST], f32, tag=tg + "in")
                for m in range(4):  # r0 r1 z0 z1
                    extra = None
                    if mrow is not None and m >= 2:
                        extra = (neg30[0:1, :], mrow)
                    gate_mm(rz[:, m * ST:m * ST + N], wihT, whhT, x_chunks, h_chunks, m,
                            rzb[0:1, m * 128:(m + 1) * 128], extra)
                for m in range(2):  # hn halves (Whh side only)
                    dst = hn[:, m * ST:m * ST + N]
                    first = True
                    for kc, hc in enumerate(h_chunks):
                        nc.tensor.matmul(dst, whhT[:, kc * 768 + (m + 4) * 128: kc * 768 + (m + 5) * 128],
                                         hc, start=first, stop=False)
                        first = False
                    nc.tensor.matmul(dst, hnb[0:1, m * 128:(m + 1) * 128],
                                     ones[0:1, 0:N], start=first, stop=True)
                for m in range(2):  # inn halves (Wih side only)
                    dst = inn[:, m * ST:m * ST + N]
                    first = True
                    for kc, xc in enumerate(x_chunks):
                        nc.tensor.matmul(dst, wihT[:, kc * 768 + (m + 4) * 128: kc * 768 + (m + 5) * 128],
                                         xc, start=first, stop=False)
                        first = False
                    nc.tensor.matmul(dst, inb[0:1, m * 128:(m + 1) * 128],
                                     ones[0:1, 0:N], start=first, stop=True)
                # gates
                rzs = wp.tile([128, 4 * N], f32, tag=tg + "rzs")
                nc.scalar.activation(rzs[:].rearrange("p (a n) -> p a n", n=N),
                                     rz[:].rearrange("p (a s) -> p a s", s=ST)[:, :, 0:N], SIG)
                hnbs = wp.tile([128, 2 * N], f32, tag=tg + "hnbs")
                nc.scalar.activation(hnbs[:].rearrange("p (a n) -> p a n", n=N),
                                     hn[:].rearrange("p (a s) -> p a s", s=ST)[:, :, 0:N], CPY)
                inns = wp.tile([128, 2 * N], f32, tag=tg + "inns")
                nc.vector.tensor_copy(inns[:].rearrange("p (a n) -> p a n", n=N),
                                      inn[:].rearrange("p (a s) -> p a s", s=ST)[:, :, 0:N])
                t1 = wp.tile([128, 2 * N], f32, tag=tg + "tmp")
                nc.vector.tensor_tensor(out=t1[:], in0=rzs[:, 0:2 * N], in1=hnbs[:], op=MUL)
                t2 = wp.tile([128, 2 * N], f32, tag=tg + "tmp")
                nc.vector.tensor_tensor(out=t2[:], in0=t1[:], in1=inns[:], op=ADD)
                ns = wp.tile([128, 2 * N], f32, tag=tg + "ns")
                nc.scalar.activation(ns[:], t2[:], TANH)
                d = wp.tile([128, 2 * N], f32, tag=tg + "tmp")
                nc.vector.tensor_tensor(out=d[:], in0=h_merged, in1=ns[:], op=SUB)
                zd = wp.tile([128, 2 * N], f32, tag=tg + "tmp")
                nc.vector.tensor_tensor(out=zd[:], in0=rzs[:, 2 * N:4 * N], in1=d[:], op=MUL)
                nc.vector.tensor_tensor(out=out_ap, in0=ns[:], in1=zd[:], op=ADD)

            def transpose_to_dram(src, half_off, n_rows, out_dram):
                """src [128, *] feature-major halves at col offsets half_off[0/1] -> out [n_rows, 256]."""
                nchunks = (n_rows + 127) // 128
                for c in range(nchunks):
                    ps = pp.tile([128, 256], mdt, tag="grz")
                    for half in range(2):
                        nc.tensor.transpose(ps[:, half * 128:(half + 1) * 128],
                                            src[:, half_off[half] + c * 128: half_off[half] + (c + 1) * 128],
                                            ident_r[:])
                    stg = wp.tile([128, 256], f32, tag="outstg")
                    nc.vector.tensor_copy(stg[:], ps[:])
                    rows = min(128, n_rows - c * 128)
                    nc.sync.dma_start(out_dram[c * 128:c * 128 + rows, :], stg[0:rows, :])

            # ================= phase 0: response prep (gather + transpose) ============
            rg = cp.tile([128, 7, 256], f32, tag="rgather")
            nc.gpsimd.dma_gather(rg[:], emb_r[:], ridx[:], NRTP, NRTP, 256)
            for c in range(7):
                ps = pp.tile([128, 256], f32, tag="ghn")
                for half in range(2):
                    nc.tensor.transpose(ps[:, half * 128:(half + 1) * 128],
                                        rg[:, c, half * 128:(half + 1) * 128], ident[:])
                for half in range(2):
                    nc.vector.tensor_copy(resp_xT[:, half * NRTP + c * 128: half * NRTP + (c + 1) * 128],
                                          ps[:, half * 128:(half + 1) * 128])

            # ================= phase 1: utterance GRU over T=50 =======================
            ucfg = {"N": NU, "st": 512, "tag": "g", "g": "u"}
            xg_tiles = []
            for t in range(2):
                g = wp.tile([128, 4, 256], f32, tag="xg")
                nc.gpsimd.dma_gather(g[:], emb_u[:], uidx[:, t * 32:(t + 1) * 32], 512, 512, 256)
                xg_tiles.append(g)
            h_prev = zeros800
            for t in range(T):
                g = xg_tiles[t % 2]
                if t + 2 < T:
                    g2 = wp.tile([128, 4, 256], f32, tag="xg")
                    nc.gpsimd.dma_gather(g2[:], emb_u[:], uidx[:, (t + 2) * 32:(t + 3) * 32], 512, 512, 256)
                    xg_tiles[t % 2] = g2
                # transpose gathered x -> xT [128, 1024] halves @0/512
                xps = pp.tile([128, 1024], f32, tag="ghn")
                for c in range(4):
                    for half in range(2):
                        nc.tensor.transpose(xps[:, half * 512 + c * 128: half * 512 + (c + 1) * 128],
                                            g[:, c, half * 128:(half + 1) * 128], ident[:])
                xT = wp.tile([128, 1024], mdt, tag="xT")
                nc.vector.tensor_copy(xT[:], xps[:])
                x_chunks = [xT[:, 0:NU], xT[:, 512:512 + NU]]
                h_chunks = [h_prev[:, 0:NU], h_prev[:, 512:512 + NU]]
                hm = h_prev[:].rearrange("p (a s) -> p a s", s=512)[:, :, 0:NU]
                if t == T - 1:
                    out_t = utterh
                else:
                    out_t = wp.tile([128, 1024], mdt, tag="uh")
                out_ap = out_t[:].rearrange("p (a s) -> p a s", s=512)[:, :, 0:NU]
                gru_step(ucfg, x_chunks, hm, h_chunks, out_ap)
                h_prev = out_t

            # ================= phase 2: ctx + resp scans (interleaved) ================
            ccfg = {"N": BL, "st": 128, "tag": "g", "g": "c"}
            rcfg = {"N": NRS, "st": 128, "tag": "g", "g": "r"}
            for s in range(S):
                # --- ctx step ---
                if s == 0:
                    hm_c = zeros800[:].rearrange("p (a s) -> p a s", s=512)[:, :, 0:BL]
                    hc_c = [zeros800[:, 0:BL], zeros800[:, 512:512 + BL]]
                else:
                    hm_c = ctx_out[:].rearrange("p (a q) -> p a q", q=512)[:, :, s - 1:400:50]
                    hc_c = [ctx_out[:, s - 1:400:50], ctx_out[:, 512 + s - 1:912:50]]
                x_c = [utterh[:, s:400:50], utterh[:, 512 + s:912:50],
                       aeT[:, s:400:50], aeT[:, 512 + s:912:50]]
                out_c = ctx_out[:].rearrange("p (a q) -> p a q", q=512)[:, :, s:400:50]
                gru_step(ccfg, x_c, hm_c, hc_c, out_c)
                # --- resp step ---
                if s == 0:
                    hm_r = zeros800[:].rearrange("p (a s) -> p a s", s=512)[:, :, 0:NRS]
                    hc_r = [zeros800[:, 0:NRS], zeros800[:, 512:512 + NRS]]
                else:
                    hm_r = resp_outs[:].rearrange("p (a q) -> p a q", q=NRTP)[:, :, (s - 1) * 16:s * 16]
                    hc_r = [resp_outs[:, (s - 1) * 16:s * 16], resp_outs[:, NRTP + (s - 1) * 16:NRTP + s * 16]]
                x_r = [resp_xT[:, s * 16:(s + 1) * 16], resp_xT[:, NRTP + s * 16:NRTP + (s + 1) * 16]]
                out_r = resp_outs[:].rearrange("p (a q) -> p a q", q=NRTP)[:, :, s * 16:(s + 1) * 16]
                gru_step(rcfg, x_r, hm_r, hc_r, out_r)

            # ================= phase 3: speaker GRU (dense, z-masked) =================
            scfg = {"N": NSPK, "st": 512, "tag": "g", "g": "s"}
            h_prev = None
            for t in range(T):
                mrow = wp.tile([1, NSPK], mdt, tag="mrow")
                nc.gpsimd.dma_start(mrow[:], mask_d[t:t + 1, :])
                scfg["mask_row"] = mrow[0:1, :]
                xb = wp.tile([128, 1024], mdt, tag="xb")
                for half in range(2):
                    nc.vector.tensor_copy(
                        xb[:, half * 512: half * 512 + NSPK].rearrange("p (b s) -> p b s", s=S + 1),
                        ctx_out[:, half * 512 + t: half * 512 + 400: 50].to_broadcast([128, BL, S + 1]))
                x_s = [xb[:, 0:NSPK], xb[:, 512:512 + NSPK]]
                if h_prev is None:
                    hm_s = zeros800[:].rearrange("p (a s) -> p a s", s=512)[:, :, 0:NSPK]
                    hc_s = [zeros800[:, 0:NSPK], zeros800[:, 512:512 + NSPK]]
                else:
                    hm_s = h_prev[:].rearrange("p (a s) -> p a s", s=512)[:, :, 0:NSPK]
                    hc_s = [h_prev[:, 0:NSPK], h_prev[:, 512:512 + NSPK]]
                out_t = wp.tile([128, 1024], mdt, tag="sh")
                out_ap = out_t[:].rearrange("p (a s) -> p a s", s=512)[:, :, 0:NSPK]
                gru_step(scfg, x_s, hm_s, hc_s, out_ap)
                h_prev = out_t

            # ================= phase 4: outputs ======================================
            transpose_to_dram(ctx_out, (0, 512), NU, ctxT_o)
            transpose_to_dram(resp_outs, (0, NRTP), NRT, respT_o)
            transpose_to_dram(h_prev, (0, 512), NSPK, spkT_o)

    nc.compile()
    return nc


def _host_inputs(inputs):
    ctx = np.asarray(inputs["context"])
    rsp = np.asarray(inputs["response"])
    spk = np.asarray(inputs["spk_agents"])
    tables = _agent_tables()
    base = {
        "emb_utter": np.asarray(inputs["emb_utter"], np.float32),
        "emb_resp": np.asarray(inputs["emb_resp"], np.float32),
    }
    for g, pre in (("u", "utter"), ("c", "ctx"), ("r", "resp"), ("s", "spk")):
        wih = np.asarray(inputs[pre + "_Wih"], np.float32)
        whh = np.asarray(inputs[pre + "_Whh"], np.float32)
        bih = np.asarray(inputs[pre + "_bih"], np.float32)
        bhh = np.asarray(inputs[pre + "_bhh"], np.float32)
        base[g + "_wihT"] = _pack_wT(wih)
        base[g + "_whhT"] = _pack_wT(whh)
        rzb = (bih + bhh)[:512].copy()
        if g == "s":
            rzb[256:512] += 30.0
        base[g + "_rzb"] = rzb[None, :]
        base[g + "_hnb"] = bhh[None, 512:].copy()
        base[g + "_inb"] = bih[None, 512:].copy()
    in_maps = []
    for c in range(NCORES):
        b0 = c * BL
        m = dict(base)
        ci = ctx[b0:b0 + BL]                          # [8, 50, 50]
        ut = ci.transpose(2, 0, 1).reshape(T, NU)     # [t][n=b*50+s]
        up = np.zeros((T, 512), np.int16)
        up[:, :NU] = ut.astype(np.int16)
        m["uidx"] = np.ascontiguousarray(np.tile(up.reshape(T, 32, 16).transpose(2, 0, 1).reshape(16, T * 32), (8, 1)))
        ri = rsp[b0:b0 + BL]                          # [8, 2, 50]
        rt = ri.transpose(2, 0, 1).reshape(NRT)       # i = t*16 + b*2 + r
        m["ridx"] = _pack_idx(rt, NRTP)
        sp = spk[b0:b0 + BL]                          # [8, 50]
        ae = tables[np.repeat(np.arange(b0, b0 + BL), S), sp.reshape(-1), :]  # [400, 256]
        m["aeT"] = np.ascontiguousarray(ae.T)
        msk = np.zeros((T, NSPK), np.float32)
        for b in range(BL):
            msk[np.arange(T), b * (S + 1) + sp[b]] = 1.0
        m["mask"] = msk
        in_maps.append(m)
    return in_maps


def kernel(**inputs):
    from concourse.bass_utils import run_bass_kernel_spmd
    if "nc" not in _cached:
        _cached["nc"] = _build_nc()
    nc = _cached["nc"]
    in_maps = _host_inputs(inputs)
    res = run_bass_kernel_spmd(nc, in_maps, core_ids=list(range(NCORES)))
    _cached["last_res"] = res

    ctx_l, resp_l, spk_l = [], [], []
    for c in range(NCORES):
        o = res.results[c]
        ctx_l.append(o["ctxT"].reshape(BL, S, H))
        resp_l.append(o["respT"].reshape(T, BL, R, H))
        spk_l.append(o["spkT"].reshape(BL, S + 1, H))
    context_output = np.concatenate(ctx_l, 0).transpose(1, 0, 2).astype(np.float32)      # [S, B, H]
    context_hidden = context_output[S - 1][None]
    resp_full = np.concatenate(resp_l, 1)                                                # [T, B, R, H]
    resp_output = resp_full.transpose(2, 0, 1, 3).astype(np.float32)                     # [R, T, B, H]
    resp_hidden = resp_output[:, T - 1]
    spk_emb = np.concatenate(spk_l, 0).astype(np.float32)                                # [B, S+1, H]
    spk = np.asarray(inputs["spk_agents"])
    sp_ids = np.arange(S + 1)
    present = (spk[:, :, None] == sp_ids[None, None, :]).any(axis=1)
    spk_emb_mask = (present & (sp_ids[None, :] > 0)).astype(np.float32)
    return (context_output, context_hidden, resp_output, resp_hidden, spk_emb, spk_emb_mask)
